# revision 1
# baseline (speedup 1.0000x reference)
import numpy as np
import jax
import jax.numpy as jnp

# ---- hardcoded problem dims (nn_LucidRains_44667659878882) ----
N_BALLS, BALL, DIM, HEADS = 16, 512, 256, 8
DH = DIM // HEADS            # 32
WIN = BALL // 16             # 32
CBS = BALL // 16             # 32
CST = BALL // 32             # 16
FBS = BALL // 16             # 32
NCB = (BALL - CBS) // CST + 1  # 31
NFB = BALL // FBS              # 16
NB = BALL // WIN               # 16
SCALE = DH ** -0.5
NDEV = 8
BPC = N_BALLS // NDEV          # 2 balls per core

# ---- static index maps / masks (baked at trace time) ----
_qi = np.arange(BALL)
WIDX = np.arange(NCB)[:, None] * CST + np.arange(CBS)[None, :]
_blk_end = np.arange(NCB) * CST + CBS - 1
CMASK = np.concatenate(
    [np.ones((BALL, 1), bool), _qi[:, None] >= _blk_end[None, :]], axis=1
)
OVERLAP = np.zeros((NCB, NFB), np.float32)
for _j in range(NCB):
    _a, _b = _j * CST, _j * CST + CBS
    for _f in range(NFB):
        _fa, _fb = _f * FBS, (_f + 1) * FBS
        OVERLAP[_j, _f] = max(0, min(_b, _fb) - max(_a, _fa)) / CBS
SELMASK = np.arange(NFB)[None, :] < (_qi // FBS)[:, None]
HAS_SEL = (_qi // FBS) > 0
OWN_TRI = (_qi % FBS)[:, None] >= np.arange(FBS)[None, :]
_qq = np.arange(WIN)[None, :, None]
_kk = np.arange(2 * WIN)[None, None, :]
_bb = np.arange(NB)[:, None, None]
SWMASK = (_kk > _qq) & (_kk <= _qq + WIN) & ((_bb > 0) | (_kk >= WIN))
NEG = -jnp.inf


def _rot(x, tpos):
    d2 = DH // 2
    inv = 1.0 / (10000.0 ** (jnp.arange(d2, dtype=jnp.float32) / d2))
    ang = tpos[:, None] * inv[None, :]
    cos, sin = jnp.cos(ang), jnp.sin(ang)
    x1, x2 = x[..., 0::2], x[..., 1::2]
    return jnp.stack((x1 * cos - x2 * sin, x1 * sin + x2 * cos), axis=-1).reshape(x.shape)


def _gmlp(xw, w1, w2):
    f = xw.reshape(xw.shape[0], HEADS, NCB, CBS * DH)
    h = jax.nn.relu(jnp.einsum('nhbi,hio->nhbo', f, w1, preferred_element_type=jnp.float32))
    return jnp.einsum('nhbi,hio->nhbo', h.astype(w2.dtype), w2, preferred_element_type=jnp.float32)


def _fwd(x, pos, Wpe, bpe, rms_w, Wqkv, kpos, vpos, kw1, kw2, vw1, vw2,
         mem_ck, mem_cv, Wg, Wo):
    n, m = BPC, BALL
    pb = pos.reshape(n, m, -1)
    rel = pb - pb.mean(axis=1, keepdims=True)
    hx = x.reshape(n, m, DIM) + rel @ Wpe + bpe
    hn = hx * jax.lax.rsqrt(jnp.mean(hx * hx, axis=-1, keepdims=True) + 1e-6) * rms_w
    q, k, v = jnp.split(hn @ Wqkv, 3, axis=-1)
    heads = lambda t: t.reshape(n, m, HEADS, DH).transpose(0, 2, 1, 3)
    q, k, v = heads(q), heads(k), heads(v)
    tpos = jnp.arange(m, dtype=jnp.float32)
    rq, rk = _rot(q, tpos), _rot(k, tpos)

    # compressed branch
    kw = k[:, :, WIDX] + kpos[None, :, None]
    vw = v[:, :, WIDX] + vpos[None, :, None]
    ck = _gmlp(kw.astype(kw1.dtype), kw1, kw2)
    cv = _gmlp(vw.astype(vw1.dtype), vw1, vw2)
    ckf = jnp.concatenate([jnp.broadcast_to(mem_ck[None], (n, HEADS, 1, DH)), ck], axis=2)
    cvf = jnp.concatenate([jnp.broadcast_to(mem_cv[None], (n, HEADS, 1, DH)), cv], axis=2)
    csim = jnp.einsum('nhid,nhjd->nhij', q, ckf) * SCALE
    cattn = jax.nn.softmax(jnp.where(CMASK[None, None], csim, NEG), axis=-1)
    c_out = jnp.einsum('nhij,nhjd->nhid', cattn, cvf)

    # fine selection
    imp = jnp.einsum('nhij,jf->nhif', cattn[..., 1:], jnp.asarray(OVERLAP))
    sel = jnp.argmax(jnp.where(SELMASK[None, None], imp, NEG), axis=-1)
    kb = rk.reshape(n, HEADS, NFB, FBS, DH)
    vb = v.reshape(n, HEADS, NFB, FBS, DH)
    qb = rq.reshape(n, HEADS, NFB, FBS, DH)
    own_s = (jnp.einsum('nhfqd,nhfkd->nhfqk', qb, kb) * SCALE).reshape(n, HEADS, m, FBS)
    fk = jnp.take_along_axis(kb, sel[..., None, None], axis=2)
    fv = jnp.take_along_axis(vb, sel[..., None, None], axis=2)
    sel_s = jnp.einsum('nhid,nhikd->nhik', rq, fk) * SCALE
    scores = jnp.concatenate([jnp.where(OWN_TRI[None, None], own_s, NEG),
                              jnp.where(HAS_SEL[None, None, :, None], sel_s, NEG)], axis=-1)
    fattn = jax.nn.softmax(scores, axis=-1)
    f_out = (jnp.einsum('nhfqk,nhfkd->nhfqd',
                        fattn[..., :FBS].reshape(n, HEADS, NFB, FBS, FBS), vb
                        ).reshape(n, HEADS, m, DH)
             + jnp.einsum('nhik,nhikd->nhid', fattn[..., FBS:], fv))

    # sliding window branch
    qw = rq.reshape(n, HEADS, NB, WIN, DH)
    kwb = rk.reshape(n, HEADS, NB, WIN, DH)
    vwb = v.reshape(n, HEADS, NB, WIN, DH)
    pad = ((0, 0), (0, 0), (1, 0), (0, 0), (0, 0))
    K2 = jnp.concatenate([jnp.pad(kwb, pad)[:, :, :NB], kwb], axis=3)
    V2 = jnp.concatenate([jnp.pad(vwb, pad)[:, :, :NB], vwb], axis=3)
    s_sw = jnp.einsum('nhbqd,nhbkd->nhbqk', qw, K2) * SCALE
    sattn = jax.nn.softmax(jnp.where(SWMASK[None, None], s_sw, NEG), axis=-1)
    sw_out = jnp.einsum('nhbqk,nhbkd->nhbqd', sattn, V2).reshape(n, HEADS, m, DH)

    # gated combine + output proj
    gates = jax.nn.sigmoid(hn @ Wg).reshape(n, m, HEADS, 3).transpose(0, 2, 1, 3)
    o = c_out * gates[..., 0:1] + f_out * gates[..., 1:2] + sw_out * gates[..., 2:3]
    o = o.transpose(0, 2, 1, 3).reshape(n, m, DIM)
    return (o @ Wo).reshape(n * m, DIM)


_PMAP = None


def _get_pmap():
    global _PMAP
    if _PMAP is None:
        _PMAP = jax.pmap(
            _fwd,
            in_axes=(0, 0) + (None,) * 14,
            devices=jax.devices()[:NDEV],
        )
    return _PMAP


def kernel(**inputs):
    x = np.asarray(inputs['x'], np.float32).reshape(NDEV, BPC * BALL, DIM)
    pos = np.asarray(inputs['pos'], np.float32).reshape(NDEV, BPC * BALL, 3)
    f = _get_pmap()
    out = f(x, pos,
            inputs['Wpe'], inputs['bpe'], inputs['rms_w'], inputs['Wqkv'],
            inputs['kpos'], inputs['vpos'], inputs['kw1'], inputs['kw2'],
            inputs['vw1'], inputs['vw2'], inputs['mem_ck'], inputs['mem_cv'],
            inputs['Wg'], inputs['Wo'])
    return np.asarray(out).reshape(N_BALLS * BALL, DIM)


# revision 3
# speedup vs baseline: 28.3074x; 28.3074x over previous
import numpy as np
import jax
import jax.numpy as jnp

# ---- hardcoded problem dims (nn_LucidRains_44667659878882) ----
N_BALLS, BALL, DIM, HEADS = 16, 512, 256, 8
DH = DIM // HEADS            # 32
WIN = BALL // 16             # 32
CBS = BALL // 16             # 32
CST = BALL // 32             # 16
FBS = BALL // 16             # 32
NCB = (BALL - CBS) // CST + 1  # 31
NFB = BALL // FBS              # 16
NB = BALL // WIN               # 16
SCALE = DH ** -0.5
NDEV = 8
BPC = N_BALLS // NDEV          # 2 balls per core

# ---- static index maps / masks (baked at trace time) ----
_qi = np.arange(BALL)
WIDX = np.arange(NCB)[:, None] * CST + np.arange(CBS)[None, :]
_blk_end = np.arange(NCB) * CST + CBS - 1
CMASK = np.concatenate(
    [np.ones((BALL, 1), bool), _qi[:, None] >= _blk_end[None, :]], axis=1
)
OVERLAP = np.zeros((NCB, NFB), np.float32)
for _j in range(NCB):
    _a, _b = _j * CST, _j * CST + CBS
    for _f in range(NFB):
        _fa, _fb = _f * FBS, (_f + 1) * FBS
        OVERLAP[_j, _f] = max(0, min(_b, _fb) - max(_a, _fa)) / CBS
SELMASK = np.arange(NFB)[None, :] < (_qi // FBS)[:, None]
HAS_SEL = (_qi // FBS) > 0
OWN_TRI = (_qi % FBS)[:, None] >= np.arange(FBS)[None, :]
_qq = np.arange(WIN)[None, :, None]
_kk = np.arange(2 * WIN)[None, None, :]
_bb = np.arange(NB)[:, None, None]
SWMASK = (_kk > _qq) & (_kk <= _qq + WIN) & ((_bb > 0) | (_kk >= WIN))
NEG = -jnp.inf


def _rot(x, tpos):
    d2 = DH // 2
    inv = 1.0 / (10000.0 ** (jnp.arange(d2, dtype=jnp.float32) / d2))
    ang = tpos[:, None] * inv[None, :]
    cos, sin = jnp.cos(ang), jnp.sin(ang)
    x1, x2 = x[..., 0::2], x[..., 1::2]
    return jnp.stack((x1 * cos - x2 * sin, x1 * sin + x2 * cos), axis=-1).reshape(x.shape)


def _gmlp(xw, w1, w2):
    f = xw.reshape(xw.shape[0], HEADS, NCB, CBS * DH)
    h = jax.nn.relu(jnp.einsum('nhbi,hio->nhbo', f, w1, preferred_element_type=jnp.float32))
    return jnp.einsum('nhbi,hio->nhbo', h.astype(w2.dtype), w2, preferred_element_type=jnp.float32)


def _fwd(x, pos, Wpe, bpe, rms_w, Wqkv, kpos, vpos, kw1, kw2, vw1, vw2,
         mem_ck, mem_cv, Wg, Wo):
    n, m = BPC, BALL
    pb = pos.reshape(n, m, -1)
    rel = pb - pb.mean(axis=1, keepdims=True)
    hx = x.reshape(n, m, DIM) + rel @ Wpe + bpe
    hn = hx * jax.lax.rsqrt(jnp.mean(hx * hx, axis=-1, keepdims=True) + 1e-6) * rms_w
    q, k, v = jnp.split(hn @ Wqkv, 3, axis=-1)
    heads = lambda t: t.reshape(n, m, HEADS, DH).transpose(0, 2, 1, 3)
    q, k, v = heads(q), heads(k), heads(v)
    tpos = jnp.arange(m, dtype=jnp.float32)
    rq, rk = _rot(q, tpos), _rot(k, tpos)

    # compressed branch
    kw = k[:, :, WIDX] + kpos[None, :, None]
    vw = v[:, :, WIDX] + vpos[None, :, None]
    ck = _gmlp(kw.astype(kw1.dtype), kw1, kw2)
    cv = _gmlp(vw.astype(vw1.dtype), vw1, vw2)
    ckf = jnp.concatenate([jnp.broadcast_to(mem_ck[None], (n, HEADS, 1, DH)), ck], axis=2)
    cvf = jnp.concatenate([jnp.broadcast_to(mem_cv[None], (n, HEADS, 1, DH)), cv], axis=2)
    csim = jnp.einsum('nhid,nhjd->nhij', q, ckf) * SCALE
    cattn = jax.nn.softmax(jnp.where(CMASK[None, None], csim, NEG), axis=-1)
    c_out = jnp.einsum('nhij,nhjd->nhid', cattn, cvf)

    # fine selection
    imp = jnp.einsum('nhij,jf->nhif', cattn[..., 1:], jnp.asarray(OVERLAP))
    sel = jnp.argmax(jnp.where(SELMASK[None, None], imp, NEG), axis=-1)
    kb = rk.reshape(n, HEADS, NFB, FBS, DH)
    vb = v.reshape(n, HEADS, NFB, FBS, DH)
    qb = rq.reshape(n, HEADS, NFB, FBS, DH)
    own_s = (jnp.einsum('nhfqd,nhfkd->nhfqk', qb, kb) * SCALE).reshape(n, HEADS, m, FBS)
    fk = jnp.take_along_axis(kb, sel[..., None, None], axis=2)
    fv = jnp.take_along_axis(vb, sel[..., None, None], axis=2)
    sel_s = jnp.einsum('nhid,nhikd->nhik', rq, fk) * SCALE
    scores = jnp.concatenate([jnp.where(OWN_TRI[None, None], own_s, NEG),
                              jnp.where(HAS_SEL[None, None, :, None], sel_s, NEG)], axis=-1)
    fattn = jax.nn.softmax(scores, axis=-1)
    f_out = (jnp.einsum('nhfqk,nhfkd->nhfqd',
                        fattn[..., :FBS].reshape(n, HEADS, NFB, FBS, FBS), vb
                        ).reshape(n, HEADS, m, DH)
             + jnp.einsum('nhik,nhikd->nhid', fattn[..., FBS:], fv))

    # sliding window branch
    qw = rq.reshape(n, HEADS, NB, WIN, DH)
    kwb = rk.reshape(n, HEADS, NB, WIN, DH)
    vwb = v.reshape(n, HEADS, NB, WIN, DH)
    pad = ((0, 0), (0, 0), (1, 0), (0, 0), (0, 0))
    K2 = jnp.concatenate([jnp.pad(kwb, pad)[:, :, :NB], kwb], axis=3)
    V2 = jnp.concatenate([jnp.pad(vwb, pad)[:, :, :NB], vwb], axis=3)
    s_sw = jnp.einsum('nhbqd,nhbkd->nhbqk', qw, K2) * SCALE
    sattn = jax.nn.softmax(jnp.where(SWMASK[None, None], s_sw, NEG), axis=-1)
    sw_out = jnp.einsum('nhbqk,nhbkd->nhbqd', sattn, V2).reshape(n, HEADS, m, DH)

    # gated combine + output proj
    gates = jax.nn.sigmoid(hn @ Wg).reshape(n, m, HEADS, 3).transpose(0, 2, 1, 3)
    o = c_out * gates[..., 0:1] + f_out * gates[..., 1:2] + sw_out * gates[..., 2:3]
    o = o.transpose(0, 2, 1, 3).reshape(n, m, DIM)
    return (o @ Wo).reshape(n * m, DIM)


_PMAP = None
_WNAMES = ['Wpe', 'bpe', 'rms_w', 'Wqkv', 'kpos', 'vpos', 'kw1', 'kw2',
           'vw1', 'vw2', 'mem_ck', 'mem_cv', 'Wg', 'Wo']
# v-side gMLP weights only: cv->c_out is smooth in the output, while the
# k-side (ck->cattn->imp->argmax) gates a discrete block selection where
# any precision loss can flip the argmax and cause large local errors.
_BF16 = {'vw1', 'vw2'}
_WCACHE = None  # (key, device_weights)


def _get_pmap():
    global _PMAP
    if _PMAP is None:
        _PMAP = jax.pmap(_fwd, in_axes=0, devices=jax.devices()[:NDEV])
    return _PMAP


def _put_weights(inputs):
    global _WCACHE
    key = tuple(id(inputs[n]) for n in _WNAMES)
    if _WCACHE is not None and _WCACHE[0] == key:
        return _WCACHE[1]
    devs = jax.devices()[:NDEV]
    ws = []
    for n in _WNAMES:
        w = np.asarray(inputs[n])
        if n in _BF16:
            w = jax.numpy.asarray(w, jax.numpy.bfloat16)
        ws.append(jax.device_put_replicated(np.asarray(w), devs))
    _WCACHE = (key, ws)
    return ws


def kernel(**inputs):
    devs = jax.devices()[:NDEV]
    x = np.asarray(inputs['x'], np.float32).reshape(NDEV, BPC * BALL, DIM)
    pos = np.asarray(inputs['pos'], np.float32).reshape(NDEV, BPC * BALL, 3)
    xs = jax.device_put_sharded(list(x), devs)
    ps = jax.device_put_sharded(list(pos), devs)
    ws = _put_weights(inputs)
    out = _get_pmap()(xs, ps, *ws)
    return np.asarray(out).astype(np.float32).reshape(N_BALLS * BALL, DIM)
